# revision 1
# baseline (speedup 1.0000x reference)
"""Masked cross-modal attention on 8 Trainium2 NeuronCores.

Reference math (per batch b):
    q,k,v = x @ W{q,k,v}.T   (head-major channels, H=8, Dh=64)
    s     = (q @ k.T) / 8, masked_fill(mask==0, 1e-9), softmax over keys
    out   = (att @ v) @ Wout.T

Masked positions contribute weight exp(1e-9)=1 and value v_j independent of
the query, so with U = unmasked keys, M = masked keys:
    out[t] = (sum_{j in U} e^{s_tj} v_j + sum_{j in M} v_j)
           / (sum_{j in U} e^{s_tj} + |M|)
The kernel runs attention only over gathered unmasked keys (~half) and the
masked-sum corrections are tiny host-side vectors added on-chip.

Sharding: core c -> batch c//2, head-group c%2 (4 of 8 heads). Each core
emits two partial [2048,512] outputs (one per head-pair through its Wout
slice); the host sums the four partials per batch.

Engine layout per core: PE does QKV projections (f32r), scores (bf16 Q^T/K^T,
row-pair packed), exp-weighted value sums with an indicator column for the
softmax denominator (f32r), and the output projection. ACT does all exp plus
the output-tile copies; DVE handles PSUM evacuation and the normalize chain.
QKV work for the second head-pair and Wout work for the first are drip-fed
between score/exp groups to keep ACT (the bottleneck) saturated.
"""

import sys

for _p in ("/opt/trn_rl_repo", "/root/.axon_site/_ro/trn_rl_repo"):
    if _p not in sys.path:
        sys.path.append(_p)

import numpy as np
import ml_dtypes
import concourse.bass as bass
import concourse.mybir as mybir
import concourse.tile as tile
from concourse import bacc
from concourse.bass_utils import run_bass_kernel_spmd

F32 = mybir.dt.float32
F32R = mybir.dt.float32r
BF16 = mybir.dt.bfloat16
EXP = mybir.ActivationFunctionType.Exp
ADD = mybir.AluOpType.add
MULT = mybir.AluOpType.mult

CDT = F32R                         # x / W / V / E / att operand dtype
CDT_NP = ml_dtypes.bfloat16 if CDT == BF16 else np.float32
QKDT = F32R                        # Q^T/K^T (scores operands) dtype

B, N, DIM = 4, 2048, 512
DL = 256                          # 4 heads * 64 dims per core
SCALE = 64 ** -0.5
TT = N // 512                     # 4 t-tiles of 512
TC = N // 128                     # 16 t-chunks of 128


def _build(nc, s_pad):
    n_sc = s_pad // 128

    xt = nc.dram_tensor("XT", [DIM, N], CDT, kind="ExternalInput")
    xgt = nc.dram_tensor("XGT", [DIM, s_pad], CDT, kind="ExternalInput")
    ind4 = nc.dram_tensor("IND4", [s_pad, 4], CDT, kind="ExternalInput")
    wqt = nc.dram_tensor("WQT", [DIM, DL], CDT, kind="ExternalInput")
    wkt = nc.dram_tensor("WKT", [DIM, DL], CDT, kind="ExternalInput")
    wvt = nc.dram_tensor("WVT", [DIM, DL], CDT, kind="ExternalInput")
    wot = nc.dram_tensor("WOT", [DL, DIM], CDT, kind="ExternalInput")
    corr = nc.dram_tensor("CORR", [65, 4], F32, kind="ExternalInput")
    outs = [nc.dram_tensor(f"OUT{hp}", [N, DIM], F32, kind="ExternalOutput")
            for hp in range(2)]

    with tile.TileContext(nc) as tc:
        with (
            tc.tile_pool(name="persist", bufs=1) as pp,
            tc.tile_pool(name="xpool", bufs=4) as xp,
            tc.tile_pool(name="ps512", bufs=4, space="PSUM") as ps512,
            tc.tile_pool(name="psreg", bufs=2, space="PSUM") as psreg,
            tc.tile_pool(name="epool", bufs=3) as ep,
            tc.tile_pool(name="npool", bufs=3) as np_pool,
            tc.tile_pool(name="dpool", bufs=2) as dpool,
            tc.tile_pool(name="drampool", bufs=4, space="DRAM") as drp,
            tc.tile_pool(name="ahpool", bufs=2) as ahp,
            tc.tile_pool(name="opool", bufs=4) as op,
        ):
            wq_sb = pp.tile([128, 4 * DL], CDT)
            wk_sb = pp.tile([128, 4 * DL], CDT)
            wv_sb = pp.tile([128, 4 * DL], CDT)
            wo_sb = pp.tile([128, 2 * DIM], CDT)
            corr_sb = pp.tile([65, 4], F32)
            qt_sb = pp.tile([128, 2 * N], QKDT)          # [d-chunk 2][t]
            kt_sb = pp.tile([128, 2 * s_pad], QKDT)      # [d-chunk 2][s]
            v_sb = pp.tile([128, n_sc * 4 * 65], CDT)    # [sc][h][65]
            att_pair = [pp.tile([128, N], CDT, name=f"attp{i}") for i in range(2)]

            # --- input DMAs, critical-path first (wk+xg gate the first scores) ---
            for k in range(4):
                nc.sync.dma_start(wk_sb[:, k * DL:(k + 1) * DL], wkt.ap()[k * 128:(k + 1) * 128, :])
            xg_tiles = []
            for k in range(4):
                xg = xp.tile([128, s_pad], CDT, tag="xg")
                nc.sync.dma_start(xg[:], xgt.ap()[k * 128:(k + 1) * 128, :])
                xg_tiles.append(xg)
            for k in range(4):
                nc.sync.dma_start(wq_sb[:, k * DL:(k + 1) * DL], wqt.ap()[k * 128:(k + 1) * 128, :])
            xt_tiles = [xp.tile([128, N], CDT, tag="xf", name=f"xf{k}") for k in range(4)]
            for k in range(4):
                nc.sync.dma_start(xt_tiles[k][:, 0:512], xt.ap()[k * 128:(k + 1) * 128, 0:512])
            for k in range(4):
                nc.sync.dma_start(wv_sb[:, k * DL:(k + 1) * DL], wvt.ap()[k * 128:(k + 1) * 128, :])
            nc.sync.dma_start(corr_sb[:], corr.ap())
            v_view = v_sb[:].rearrange("p (s h x) -> p s h x", s=n_sc, h=4)
            for sc in range(n_sc):
                nc.sync.dma_start(v_view[:, sc, :, 64], ind4.ap()[sc * 128:(sc + 1) * 128, :])
            for t in range(1, TT):
                for k in range(4):
                    nc.sync.dma_start(xt_tiles[k][:, t * 512:(t + 1) * 512],
                                      xt.ap()[k * 128:(k + 1) * 128, t * 512:(t + 1) * 512])
            for k in range(2):
                nc.sync.dma_start(wo_sb[:, k * DIM:(k + 1) * DIM], wot.ap()[k * 128:(k + 1) * 128, :])

            s_tiles = [(i * 512, min(512, s_pad - i * 512)) for i in range((s_pad + 511) // 512)]

            def emit_kt(dc, s0, sw):
                pk = ps512.tile([128, 512], F32, tag="ps512", name="pk")
                for k in range(4):
                    nc.tensor.matmul(
                        pk[:, :sw],
                        wk_sb[:, k * DL + dc * 128: k * DL + (dc + 1) * 128],
                        xg_tiles[k][:, s0:s0 + sw],
                        start=(k == 0), stop=(k == 3),
                    )
                nc.vector.tensor_copy(kt_sb[:, dc * s_pad + s0: dc * s_pad + s0 + sw], pk[:, :sw])

            def emit_qt(dc, t):
                pq = ps512.tile([128, 512], F32, tag="ps512", name="pq")
                for k in range(4):
                    nc.tensor.matmul(
                        pq[:],
                        wq_sb[:, k * DL + dc * 128: k * DL + (dc + 1) * 128],
                        xt_tiles[k][:, t * 512:(t + 1) * 512],
                        start=(k == 0), stop=(k == 3),
                    )
                nc.vector.tensor_copy(qt_sb[:, dc * N + t * 512: dc * N + (t + 1) * 512], pq[:])

            def emit_v(sc):
                pv = ps512.tile([128, 256], F32, tag="ps512", name="pv")
                for k in range(4):
                    nc.tensor.matmul(
                        pv[:],
                        xg_tiles[k][:, sc * 128:(sc + 1) * 128],
                        wv_sb[:, k * DL:(k + 1) * DL],
                        start=(k == 0), stop=(k == 3),
                    )
                nc.vector.tensor_copy(
                    v_view[:, sc, :, 0:64],
                    pv[:].rearrange("p (h x) -> p h x", h=4),
                )

            def emit_wout_chunk(hp, t):
                po = ps512.tile([128, 512], F32, tag="ps512", name="po")
                nc.tensor.matmul(
                    po[:],
                    att_pair[hp][:, t * 128:(t + 1) * 128],
                    wo_sb[:, hp * DIM:(hp + 1) * DIM],
                    start=True, stop=True,
                )
                o_sb = op.tile([128, 512], F32, tag="o")
                nc.scalar.copy(o_sb[:], po[:])
                nc.sync.dma_start(outs[hp].ap()[t * 128:(t + 1) * 128, :], o_sb[:])

            def emit_normalize_half(hp, h, half, numer_sb):
                # one t-half (1024 tokens) of head h: denominator -> recip -> scale
                sl = slice(half * 1024, (half + 1) * 1024)
                den = dpool.tile([65, 1024], F32, tag="den")
                nc.vector.tensor_scalar_add(
                    den[64:65, :], numer_sb[h][64:65, sl], corr_sb[64:65, h:h + 1])
                scratch = drp.tile([1024], F32, tag="scr")
                nc.sync.dma_start(scratch[:].unsqueeze(0), den[64:65, :])
                bden = dpool.tile([64, 1024], F32, tag="bden")
                nc.sync.dma_start(bden[:], scratch[:].unsqueeze(0).broadcast_to([64, 1024]))
                rbc = dpool.tile([64, 1024], F32, tag="rbc")
                nc.vector.reciprocal_approx_fast(out=rbc[:], in_=bden[:])
                att_h = ahp.tile([64, 1024], CDT, tag="att")
                nc.vector.scalar_tensor_tensor(
                    out=att_h[:], in0=numer_sb[h][0:64, sl],
                    scalar=corr_sb[0:64, h:h + 1], in1=rbc[:],
                    op0=ADD, op1=MULT,
                )
                par = (h % 2) * 64
                nc.sync.dma_start(att_pair[hp][par:par + 64, sl], att_h[:])

            # filler work drip-fed one unit per score/exp group
            fillers = []

            def drain_filler():
                if fillers:
                    fillers.pop(0)()

            # K^T d-chunk 0 gates the whole pipeline: emit first
            for s0, sw in s_tiles:
                emit_kt(0, s0, sw)

            for hp in range(2):
                numer_sb = {}
                for h in (2 * hp, 2 * hp + 1):
                    numer_sb[h] = np_pool.tile([65, N], F32, tag="numer", name=f"numer{h}")
                if hp == 1:
                    # second pair: all QKV done; fillers drain Wout of pair 0
                    fillers.extend([
                        (lambda t=t: emit_wout_chunk(0, t)) for t in range(TC)
                    ])
                for t in range(TT):
                    if hp == 0:
                        emit_qt(0, t)
                        if t == 1:
                            # drip KT dc1 then QT dc1 between upcoming groups
                            for s0, sw in s_tiles:
                                fillers.append(lambda s0=s0, sw=sw: emit_kt(1, s0, sw))
                            for tq in range(TT):
                                fillers.append(lambda tq=tq: emit_qt(1, tq))
                    pn = {}
                    for h in (2 * hp, 2 * hp + 1):
                        pn[h] = ps512.tile([65, 512], F32, tag="ps512", name=f"pn{h}")
                    for sc in range(n_sc):
                        reg = psreg.tile([128, 1024], F32, tag="reg")
                        e_sb = ep.tile([128, 1024], CDT, tag="e")
                        for j, h in enumerate((2 * hp, 2 * hp + 1)):
                            par = (h % 2) * 64
                            nc.tensor.matmul(
                                reg[:, j * 512:(j + 1) * 512],
                                kt_sb[par:par + 64, hp * s_pad + sc * 128: hp * s_pad + (sc + 1) * 128],
                                qt_sb[par:par + 64, hp * N + t * 512: hp * N + (t + 1) * 512],
                                start=True, stop=True,
                            )
                        if hp == 0 and t == 0:
                            emit_v(sc)
                        else:
                            drain_filler()
                        nc.scalar.activation(e_sb[:], reg[:], EXP, scale=SCALE)
                        for j, h in enumerate((2 * hp, 2 * hp + 1)):
                            nc.tensor.matmul(
                                pn[h][:],
                                v_sb[:, (sc * 4 + h) * 65:(sc * 4 + h + 1) * 65],
                                e_sb[:, j * 512:(j + 1) * 512],
                                start=(sc == 0), stop=(sc == n_sc - 1),
                            )
                    for h in (2 * hp, 2 * hp + 1):
                        nc.vector.tensor_copy(numer_sb[h][:, t * 512:(t + 1) * 512], pn[h][:])
                    if t == 1:
                        for h in (2 * hp, 2 * hp + 1):
                            emit_normalize_half(hp, h, 0, numer_sb)
                # finish pair: second halves
                for h in (2 * hp, 2 * hp + 1):
                    emit_normalize_half(hp, h, 1, numer_sb)
                while fillers:
                    drain_filler()
            for t in range(TC):
                emit_wout_chunk(1, t)

    nc.compile()
    return nc


def _prep(input_feature, mask, Wq, Wk, Wv, Wout):
    x = np.ascontiguousarray(np.asarray(input_feature, dtype=np.float32))
    m = np.asarray(mask)
    Wq = np.asarray(Wq, dtype=np.float32)
    Wk = np.asarray(Wk, dtype=np.float32)
    Wv = np.asarray(Wv, dtype=np.float32)
    Wout = np.asarray(Wout, dtype=np.float32)

    idxs = [np.flatnonzero(m[b]) for b in range(B)]
    s_pad = max(128, ((max(len(i) for i in idxs) + 127) // 128) * 128)

    def cvt(a):
        return np.ascontiguousarray(a.astype(CDT_NP))

    in_maps = []
    for c in range(8):
        b, g = c // 2, c % 2
        idx = idxs[b]
        cnt = len(idx)
        xg = np.zeros((s_pad, DIM), np.float32)
        xg[:cnt] = x[b][idx]
        ind4 = np.zeros((s_pad, 4), np.float32)
        ind4[:cnt] = 1.0
        xm = x[b][m[b] == 0].sum(axis=0, dtype=np.float32)
        corr = np.zeros((65, 4), np.float32)
        for h in range(4):
            hg = g * 4 + h
            corr[0:64, h] = Wv[hg * 64:(hg + 1) * 64, :] @ xm
            corr[64, h] = np.float32(N - cnt)
        in_maps.append({
            "XT": cvt(x[b].T),
            "XGT": cvt(xg.T),
            "IND4": cvt(ind4),
            "WQT": cvt(Wq[g * DL:(g + 1) * DL, :].T),
            "WKT": cvt(Wk[g * DL:(g + 1) * DL, :].T),
            "WVT": cvt(Wv[g * DL:(g + 1) * DL, :].T),
            "WOT": cvt(Wout[:, g * DL:(g + 1) * DL].T),
            "CORR": corr,
        })
    return in_maps, s_pad


def _run(in_maps, s_pad, trace=False):
    nc = bacc.Bacc("TRN2", target_bir_lowering=False, debug=False, num_devices=8)
    _build(nc, s_pad)
    res = run_bass_kernel_spmd(nc, in_maps, core_ids=list(range(8)), trace=trace)
    out = np.empty((B, N, DIM), np.float32)
    for b in range(B):
        out[b] = (res.results[2 * b]["OUT0"] + res.results[2 * b]["OUT1"]
                  + res.results[2 * b + 1]["OUT0"] + res.results[2 * b + 1]["OUT1"])
    return out, res


def kernel(input_feature, mask, Wq, Wk, Wv, Wout):
    in_maps, s_pad = _prep(input_feature, mask, Wq, Wk, Wv, Wout)
    out, _ = _run(in_maps, s_pad)
    return out



# revision 11
# speedup vs baseline: 1.1214x; 1.1214x over previous
"""Masked cross-modal attention on 8 Trainium2 NeuronCores (v3).

Reference math (per batch b):
    q,k,v = x @ W{q,k,v}.T   (head-major channels, H=8, Dh=64)
    s     = (q @ k.T) / 8, masked_fill(mask==0, 1e-9), softmax over keys
    out   = (att @ v) @ Wout.T

Masked positions contribute weight exp(1e-9)=1 and value v_j independent of
the query, so attention runs only over gathered unmasked keys (~half).  The
|M| denominator correction is folded into the indicator column of the padded
(zero) key rows: a zero key row scores 0 against every query, exp gives
exactly 1, and indicator (N-cnt)/n_pad makes the indicator matmul emit the
complete denominator.  The masked-value numerator correction is a tiny
host-side [64] vector fused into the normalize multiply.

Sharding: core c -> batch c//2, head-group c%2 (4 of 8 heads).  Each core
emits two partial [2048,512] outputs (one per head-pair through its Wout
slice); the host sums the four partials per batch.

Engine layout per core: PE does QKV projections (bf16), scores (bf16),
exp-weighted value sums with indicator column (f32r), and the output
projection (f32r), software-pipelined as scores(sc) -> attV(sc-1) so PE
always has a dependency-free matmul in flight.  ACT does only exp (f32r
out) plus the second head-pair's output-tile copies at the tail.  DVE
handles PSUM evacuation and the normalize chain.  QKV and first-pair Wout
groups are drip-fed into scheduled slots of the attention loop.
"""

import sys

for _p in ("/opt/trn_rl_repo", "/root/.axon_site/_ro/trn_rl_repo"):
    if _p not in sys.path:
        sys.path.append(_p)

import numpy as np
import ml_dtypes
import concourse.bass as bass
import concourse.mybir as mybir
import concourse.tile as tile
from concourse import bacc
from concourse.bass_utils import run_bass_kernel_spmd

F32 = mybir.dt.float32
F32R = mybir.dt.float32r
BF16 = mybir.dt.bfloat16
EXP = mybir.ActivationFunctionType.Exp
ADD = mybir.AluOpType.add
MULT = mybir.AluOpType.mult

XDT = BF16                         # x / xg / Wq / Wk / Wv / Q^T / K^T
XDT_NP = ml_dtypes.bfloat16
VDT = F32R                         # e / V / att / Wout operands
VDT_NP = np.float32
ODT = BF16                         # output partials
ODT_NP = ml_dtypes.bfloat16

B, N, DIM = 4, 2048, 512
DL = 256                          # 4 heads * 64 dims per core
SCALE = 64 ** -0.5


def _build(nc, s_pad):
    n_sc = s_pad // 128

    xt = nc.dram_tensor("XT", [DIM, N], XDT, kind="ExternalInput")
    xgt = nc.dram_tensor("XGT", [DIM, s_pad], XDT, kind="ExternalInput")
    indv = nc.dram_tensor("INDV", [s_pad, 4], VDT, kind="ExternalInput")
    wqt = nc.dram_tensor("WQT", [DIM, DL], XDT, kind="ExternalInput")
    wkt = nc.dram_tensor("WKT", [DIM, DL], XDT, kind="ExternalInput")
    wvt = nc.dram_tensor("WVT", [DIM, DL], XDT, kind="ExternalInput")
    wot = nc.dram_tensor("WOT", [DL, DIM], VDT, kind="ExternalInput")
    corr = nc.dram_tensor("CORR", [64, 4], F32, kind="ExternalInput")
    outs = [nc.dram_tensor(f"OUT{hp}", [N, DIM], ODT, kind="ExternalOutput")
            for hp in range(2)]

    s_tiles = [(i * 512, min(512, s_pad - i * 512)) for i in range((s_pad + 511) // 512)]

    with tile.TileContext(nc) as tc:
        with (
            tc.tile_pool(name="persist", bufs=1) as pp,
            tc.tile_pool(name="xpool", bufs=4) as xp,
            tc.tile_pool(name="psaux", bufs=2, space="PSUM") as psaux,
            tc.tile_pool(name="psreg", bufs=2, space="PSUM") as psreg,
            tc.tile_pool(name="pspn", bufs=1, space="PSUM") as pspn,
            tc.tile_pool(name="epool", bufs=3) as ep,
            tc.tile_pool(name="npool", bufs=2) as np_pool,
            tc.tile_pool(name="dpool", bufs=2) as dpool,
            tc.tile_pool(name="drampool", bufs=2, space="DRAM") as drp,
            tc.tile_pool(name="opool", bufs=3) as op,
        ):
            wq_sb = pp.tile([128, 4 * DL], XDT)
            wk_sb = pp.tile([128, 4 * DL], XDT)
            wv_sb = pp.tile([128, 4 * DL], XDT)
            wo_sb = pp.tile([128, 2 * DIM], VDT)
            corr_sb = pp.tile([64, 4], F32)
            qt_sb = pp.tile([128, 2 * N], XDT)           # [hp][t]
            kt_sb = pp.tile([128, 2 * s_pad], XDT)       # [hp][s]
            v_sb = pp.tile([128, n_sc * 4 * 65], VDT)    # [sc][h][65]
            att_pair = [pp.tile([128, N], VDT, name=f"attp{i}") for i in range(2)]

            # --- input DMAs, critical-path first (wk+xg gate the first scores) ---
            for k in range(4):
                nc.sync.dma_start(wk_sb[:, k * DL:(k + 1) * DL], wkt.ap()[k * 128:(k + 1) * 128, :])
            xg_tiles = []
            for k in range(4):
                xg = xp.tile([128, s_pad], XDT, tag="xg")
                nc.sync.dma_start(xg[:], xgt.ap()[k * 128:(k + 1) * 128, :])
                xg_tiles.append(xg)
            for k in range(4):
                nc.sync.dma_start(wq_sb[:, k * DL:(k + 1) * DL], wqt.ap()[k * 128:(k + 1) * 128, :])
            xt_tiles = [xp.tile([128, N], XDT, tag="xf", name=f"xf{k}") for k in range(4)]
            for k in range(4):
                nc.sync.dma_start(xt_tiles[k][:, 0:1024], xt.ap()[k * 128:(k + 1) * 128, 0:1024])
            for k in range(4):
                nc.sync.dma_start(wv_sb[:, k * DL:(k + 1) * DL], wvt.ap()[k * 128:(k + 1) * 128, :])
            nc.sync.dma_start(corr_sb[:], corr.ap())
            v_view = v_sb[:].rearrange("p (s h x) -> p s h x", s=n_sc, h=4)
            for sc in range(n_sc):
                nc.sync.dma_start(v_view[:, sc, :, 64], indv.ap()[sc * 128:(sc + 1) * 128, :])
            for k in range(4):
                nc.sync.dma_start(xt_tiles[k][:, 1024:2048],
                                  xt.ap()[k * 128:(k + 1) * 128, 1024:2048])
            for k in range(2):
                nc.sync.dma_start(wo_sb[:, k * DIM:(k + 1) * DIM], wot.ap()[k * 128:(k + 1) * 128, :])

            def emit_kt(hp, si):
                s0, sw = s_tiles[si]
                pk = psaux.tile([128, 512], F32, tag="psaux", name="pk")
                for k in range(4):
                    nc.tensor.matmul(
                        pk[:, :sw],
                        wk_sb[:, k * DL + hp * 128: k * DL + (hp + 1) * 128],
                        xg_tiles[k][:, s0:s0 + sw],
                        start=(k == 0), stop=(k == 3),
                    )
                nc.vector.tensor_copy(kt_sb[:, hp * s_pad + s0: hp * s_pad + s0 + sw], pk[:, :sw])

            def emit_qt(hp, t):
                pq = psaux.tile([128, 512], F32, tag="psaux", name="pq")
                for k in range(4):
                    nc.tensor.matmul(
                        pq[:],
                        wq_sb[:, k * DL + hp * 128: k * DL + (hp + 1) * 128],
                        xt_tiles[k][:, t * 512:(t + 1) * 512],
                        start=(k == 0), stop=(k == 3),
                    )
                nc.vector.tensor_copy(qt_sb[:, hp * N + t * 512: hp * N + (t + 1) * 512], pq[:])

            def emit_v(sc):
                pv = psaux.tile([128, 256], F32, tag="psaux", name="pv")
                for k in range(4):
                    nc.tensor.matmul(
                        pv[:],
                        xg_tiles[k][:, sc * 128:(sc + 1) * 128],
                        wv_sb[:, k * DL:(k + 1) * DL],
                        start=(k == 0), stop=(k == 3),
                    )
                nc.vector.tensor_copy(
                    v_view[:, sc, :, 0:64],
                    pv[:].rearrange("p (h x) -> p h x", h=4),
                )

            def emit_wout(hp, tcn):
                po = psaux.tile([128, 512], F32, tag="psaux", name="po")
                nc.tensor.matmul(
                    po[:],
                    att_pair[hp][:, tcn * 128:(tcn + 1) * 128],
                    wo_sb[:, hp * DIM:(hp + 1) * DIM],
                    start=True, stop=True,
                )
                o_sb = op.tile([128, 512], ODT, tag="o")
                if hp == 0 or tcn < 8:
                    nc.vector.tensor_copy(o_sb[:], po[:])
                else:
                    nc.scalar.copy(o_sb[:], po[:])
                nc.sync.dma_start(outs[hp].ap()[tcn * 128:(tcn + 1) * 128, :], o_sb[:])

            def emit_normalize(hp, hl, half, numer):
                gh = 2 * hp + hl
                scratch = drp.tile([1024], F32, tag="scr")
                nc.sync.dma_start(scratch[:].unsqueeze(0), numer[64:65, :])
                bden = dpool.tile([64, 1024], F32, tag="bden")
                nc.sync.dma_start(bden[:], scratch[:].unsqueeze(0).broadcast_to([64, 1024]))
                rbc = dpool.tile([64, 1024], F32, tag="rbc")
                nc.vector.reciprocal_approx_fast(out=rbc[:], in_=bden[:])
                par = hl * 64
                nc.vector.scalar_tensor_tensor(
                    out=att_pair[hp][par:par + 64, half * 1024:(half + 1) * 1024],
                    in0=numer[0:64, :],
                    scalar=corr_sb[:, gh:gh + 1],
                    in1=rbc[:],
                    op0=ADD, op1=MULT,
                )

            # fillers[unit][slot] -> list of closures to emit in that sc slot;
            # unit = (hp, hl, half) flattened 0..7, slot = sc index 1..n_sc-1
            fillers = {}

            def add_fill(unit, slot, fn):
                fillers.setdefault((unit, slot), []).append(fn)

            # unit 0 (h0-half0): rest of kt hp0, v4.., qt02/03 before half1
            add_fill(0, 1, lambda: emit_kt(0, 1))
            if len(s_tiles) > 2:
                add_fill(0, 2, lambda: emit_kt(0, 2))
            for sc in range(4, n_sc):
                add_fill(0, min(sc - 1, n_sc - 2), lambda sc=sc: emit_v(sc))
            add_fill(0, n_sc - 2, lambda: emit_qt(0, 2))
            add_fill(0, n_sc - 1, lambda: emit_qt(0, 3))
            # unit 1 (h0-half1): kt/qt of hp1
            for i, _ in enumerate(s_tiles):
                add_fill(1, 1 + i, lambda i=i: emit_kt(1, i))
            for t in range(4):
                add_fill(1, 4 + t, lambda t=t: emit_qt(1, t))
            # units 4,5 (h2): Wout hp0 chunks
            for i in range(16):
                add_fill(4 + i // 8, 1 + (i % 8), lambda i=i: emit_wout(0, i))
            # unit 7 (h3-half1): first Wout hp1 chunks once h3-half0's
            # normalize chain (~5us latency) has surely landed
            for i in range(2):
                add_fill(7, 7 + i, lambda i=i: emit_wout(1, i))

            # head start: first kt window, first q half, v0..v3
            emit_kt(0, 0)
            emit_qt(0, 0)
            emit_qt(0, 1)
            for sc in range(4):
                emit_v(sc)

            for hp in range(2):
                for hl in range(2):
                    gh = 2 * hp + hl
                    par = hl * 64
                    for half in range(2):
                        unit = ((hp * 2) + hl) * 2 + half
                        pn = pspn.tile([65, 1024], F32, tag="pn")
                        e_tiles = {}
                        for sc in range(n_sc):
                            reg = psreg.tile([128, 1024], F32, tag="reg")
                            for j in range(2):
                                nc.tensor.matmul(
                                    reg[:, j * 512:(j + 1) * 512],
                                    kt_sb[par:par + 64, hp * s_pad + sc * 128: hp * s_pad + (sc + 1) * 128],
                                    qt_sb[par:par + 64,
                                          hp * N + half * 1024 + j * 512: hp * N + half * 1024 + (j + 1) * 512],
                                    start=True, stop=True,
                                )
                            e_sb = ep.tile([128, 1024], VDT, tag="e")
                            nc.scalar.activation(e_sb[:], reg[:], EXP, scale=SCALE)
                            e_tiles[sc] = e_sb
                            for fn in fillers.pop((unit, sc), ()):
                                fn()
                            if sc > 0:
                                prev = e_tiles.pop(sc - 1)
                                for j in range(2):
                                    nc.tensor.matmul(
                                        pn[:, j * 512:(j + 1) * 512],
                                        v_sb[:, ((sc - 1) * 4 + gh) * 65:((sc - 1) * 4 + gh + 1) * 65],
                                        prev[:, j * 512:(j + 1) * 512],
                                        start=(sc - 1 == 0), stop=False,
                                    )
                        last = e_tiles.pop(n_sc - 1)
                        for j in range(2):
                            nc.tensor.matmul(
                                pn[:, j * 512:(j + 1) * 512],
                                v_sb[:, ((n_sc - 1) * 4 + gh) * 65:((n_sc - 1) * 4 + gh + 1) * 65],
                                last[:, j * 512:(j + 1) * 512],
                                start=False, stop=True,
                            )
                        numer = np_pool.tile([65, 1024], F32, tag="numer")
                        nc.vector.tensor_copy(numer[:], pn[:])
                        emit_normalize(hp, hl, half, numer)
            assert not fillers, f"unconsumed fillers: {list(fillers)}"
            for tcn in range(2, 16):
                emit_wout(1, tcn)

    nc.compile()
    return nc


def _prep(input_feature, mask, Wq, Wk, Wv, Wout):
    x = np.ascontiguousarray(np.asarray(input_feature, dtype=np.float32))
    m = np.asarray(mask)
    Wq = np.asarray(Wq, dtype=np.float32)
    Wk = np.asarray(Wk, dtype=np.float32)
    Wv = np.asarray(Wv, dtype=np.float32)
    Wout = np.asarray(Wout, dtype=np.float32)

    idxs = [np.flatnonzero(m[b]) for b in range(B)]
    max_cnt = max(len(i) for i in idxs)
    s_pad = max(128, ((max_cnt + 127) // 128) * 128)
    if s_pad == max_cnt:
        s_pad += 128  # every batch needs >=1 phantom key row

    in_maps = []
    for c in range(8):
        b, g = c // 2, c % 2
        idx = idxs[b]
        cnt = len(idx)
        xg = np.zeros((s_pad, DIM), np.float32)
        xg[:cnt] = x[b][idx]
        n_pad = s_pad - cnt
        iv = np.zeros((s_pad, 4), np.float32)
        iv[:cnt] = 1.0
        iv[cnt:] = np.float32(N - cnt) / np.float32(n_pad)
        xm = x[b][m[b] == 0].sum(axis=0, dtype=np.float32)
        corr = np.zeros((64, 4), np.float32)
        for h in range(4):
            hg = g * 4 + h
            corr[:, h] = Wv[hg * 64:(hg + 1) * 64, :] @ xm
        in_maps.append({
            "XT": np.ascontiguousarray(x[b].T.astype(XDT_NP)),
            "XGT": np.ascontiguousarray(xg.T.astype(XDT_NP)),
            "INDV": np.ascontiguousarray(iv.astype(VDT_NP)),
            "WQT": np.ascontiguousarray(Wq[g * DL:(g + 1) * DL, :].T.astype(XDT_NP)),
            "WKT": np.ascontiguousarray(Wk[g * DL:(g + 1) * DL, :].T.astype(XDT_NP)),
            "WVT": np.ascontiguousarray(Wv[g * DL:(g + 1) * DL, :].T.astype(XDT_NP)),
            "WOT": np.ascontiguousarray(Wout[:, g * DL:(g + 1) * DL].T.astype(VDT_NP)),
            "CORR": corr,
        })
    return in_maps, s_pad


def _run(in_maps, s_pad, trace=False):
    nc = bacc.Bacc("TRN2", target_bir_lowering=False, debug=False, num_devices=8)
    _build(nc, s_pad)
    res = run_bass_kernel_spmd(nc, in_maps, core_ids=list(range(8)), trace=trace)
    out = np.empty((B, N, DIM), np.float32)
    for b in range(B):
        out[b] = (res.results[2 * b]["OUT0"].astype(np.float32)
                  + res.results[2 * b]["OUT1"].astype(np.float32)
                  + res.results[2 * b + 1]["OUT0"].astype(np.float32)
                  + res.results[2 * b + 1]["OUT1"].astype(np.float32))
    return out, res


def kernel(input_feature, mask, Wq, Wk, Wv, Wout):
    in_maps, s_pad = _prep(input_feature, mask, Wq, Wk, Wv, Wout)
    out, _ = _run(in_maps, s_pad)
    return out
